# revision 30
# baseline (speedup 1.0000x reference)
"""Multi-head attention (B=4, N=2048, C=1024, H=16) on 8 TRN2 NeuronCores.

Sharding: (batch, head-group) pairs -> 8 cores. Core c handles batch c//2 and
heads [ (c%2)*8 , (c%2)*8+8 ). QKV weights are column-sharded per head group,
proj weights row-sharded; each core emits a partial proj output (transposed),
and the host sums the two partials per batch and adds b_proj.

Device dataflow per core (all matmuls contract over the SBUF partition dim):
  x[b].T (host-prepped)  ->  qk^T = [wq|wk]^T-style matmul (scale folded into q)
  v in natural [n, d] layout, stored as [v | 1 | pad] per head (128 cols, FWL)
  scores^T[k, q] = (k^T chunk)^T-stationary @ q^T     (no transposes needed)
  p^T = exp(scores^T)                                  (safe: |scores| < ~3)
  [out^T; Z] = [v|1]^T-stationary @ p^T                (Z row = softmax denom)
  out^T *= broadcast(1/Z)                              (gpsimd partition bcast)
  y^T[c2, n] = w_proj-stationary @ out^T               (partial over local heads)
"""

import os
import sys

import numpy as np

for _p in ("/root/.axon_site", "/root/.axon_site/_ro/trn_rl_repo",
           "/root/.axon_site/_ro/pypackages", "/opt/trn_rl_repo", "/opt/pypackages"):
    if os.path.isdir(_p) and _p not in sys.path:
        sys.path.append(_p)

import concourse.bacc as bacc
import concourse.mybir as mybir
import concourse.tile as tile
from concourse.bass_utils import run_bass_kernel_spmd

B, N, C = 4, 2048, 1024
H, D = 16, 64
NH = 8            # heads per core
CL = NH * D       # 512 local channels
NCORES = 8
SCALE = float(D) ** -0.5

F32 = mybir.dt.float32
BF16 = mybir.dt.bfloat16

# matmul compute dtype: "bf16" (1 cyc/row) or "f32" (4 cyc/row) or "f32r"
MM_DT = os.environ.get("ATTN_MM_DT", "bf16")

# Schraudolph exp constants (DVE bit-trick offload for part of the softmax)
EXP_A = float(2 ** 23 / np.log(2.0))
EXP_B = float(127 * 2 ** 23 - 361009)

_CACHE = {}


def _mm(nc, out, lhsT, rhs, start, stop, dt):
    if dt == "f32r":
        lhsT = lhsT.bitcast(mybir.dt.float32r)
        rhs = rhs.bitcast(mybir.dt.float32r)
    nc.tensor.matmul(out, lhsT, rhs, start=start, stop=stop)


def build_nc(mm_dt=MM_DT):
    sb_dt = BF16 if mm_dt == "bf16" else F32
    nc = bacc.Bacc()

    xT = nc.declare_dram_parameter("xT", [C, N], sb_dt, isOutput=False)
    wqk = nc.declare_dram_parameter("wqk", [C, 2 * CL], sb_dt, isOutput=False)
    wv = nc.declare_dram_parameter("wv", [C, CL], sb_dt, isOutput=False)
    wp = nc.declare_dram_parameter("wp", [CL, C], sb_dt, isOutput=False)
    bqk = nc.declare_dram_parameter("bqk", [128, 8], F32, isOutput=False)
    bv = nc.declare_dram_parameter("bv", [128, CL], F32, isOutput=False)
    yT = nc.declare_dram_parameter("yT", [C, N], F32, isOutput=True)

    Ident = mybir.ActivationFunctionType.Identity
    Exp = mybir.ActivationFunctionType.Exp
    Mult = mybir.AluOpType.mult
    Add = mybir.AluOpType.add

    with tile.TileContext(nc) as tc:
        with (
            tc.tile_pool(name="const", bufs=1) as const,
            tc.tile_pool(name="wpool", bufs=1) as wpool,
            tc.tile_pool(name="qkpool", bufs=1) as qkpool,
            tc.tile_pool(name="vpool", bufs=1) as vpool,
            tc.tile_pool(name="aopool", bufs=1) as aopool,
        ):
            bqk_t = const.tile([128, 8], F32, tag="bqk")
            nc.sync.dma_start(out=bqk_t[:], in_=bqk[:])
            bv_t = const.tile([128, CL], F32, tag="bv")
            nc.sync.dma_start(out=bv_t[:], in_=bv[:])

            wqk_t = []
            wv_t = []
            for cc in range(8):
                wt = wpool.tile([128, 2 * CL], sb_dt, tag=f"wqk{cc}", name=f"wqk{cc}")
                nc.sync.dma_start(out=wt[:], in_=wqk[cc * 128:(cc + 1) * 128, :])
                wqk_t.append(wt)
                vt = wpool.tile([128, CL], sb_dt, tag=f"wv{cc}", name=f"wv{cc}")
                nc.sync.dma_start(out=vt[:], in_=wv[cc * 128:(cc + 1) * 128, :])
                wv_t.append(vt)
            wp_t = []
            for cl in range(4):
                wt = wpool.tile([128, C], sb_dt, tag=f"wp{cl}", name=f"wp{cl}")
                nc.sync.dma_start(out=wt[:], in_=wp[cl * 128:(cl + 1) * 128, :])
                wp_t.append(wt)

            # persistent intermediates
            qk_t = []   # qk^T tiles: m 0..3 -> q^T (pre-scaled), 4..7 -> k^T
            for m in range(8):
                t = qkpool.tile([128, N], sb_dt, tag=f"qk{m}", name=f"qk{m}")
                qk_t.append(t)
            v_t = []    # [v | 1] per head: 8 groups of 65 cols
            for i in range(16):
                t = vpool.tile([128, NH * 128], sb_dt, tag=f"v{i}", name=f"v{i}")
                v_t.append(t)
            ao_t = []   # attention output^T (normalized), 4 chunks of 128 ch
            for cl in range(4):
                t = aopool.tile([128, N], sb_dt, tag=f"ao{cl}", name=f"ao{cl}")
                ao_t.append(t)

            with tc.tile_pool(name="xpool", bufs=1) as xpool, \
                 tc.tile_pool(name="psAB", bufs=6, space="PSUM") as psAB:
                xT_t = []
                for cc in range(8):
                    t = xpool.tile([128, N], sb_dt, tag=f"xT{cc}", name=f"xT{cc}")
                    nc.sync.dma_start(out=t[:], in_=xT[cc * 128:(cc + 1) * 128, :])
                    xT_t.append(t)

                # ---- phase A/B: qk^T, v. Order: k-tiles, q(j=0), v, then
                # q(j=1..3) so attention can start early and overlap. ----
                def qk_unit(m, j):
                    ps = psAB.tile([128, 512], F32, tag="ps", name="ps")
                    for cc in range(8):
                        _mm(nc, ps[:],
                            wqk_t[cc][:, m * 128:(m + 1) * 128],
                            xT_t[cc][:, j * 512:(j + 1) * 512],
                            cc == 0, cc == 7, mm_dt)
                    nc.scalar.activation(
                        qk_t[m][:, j * 512:(j + 1) * 512], ps[:], Ident,
                        bias=bqk_t[:, m:m + 1],
                        scale=SCALE if m < 4 else 1.0)

                def v_unit(i):
                    ps = psAB.tile([128, 512], F32, tag="ps", name="ps")
                    for cc in range(8):
                        _mm(nc, ps[:],
                            xT_t[cc][:, i * 128:(i + 1) * 128],
                            wv_t[cc][:],
                            cc == 0, cc == 7, mm_dt)
                    v3 = v_t[i].rearrange("p (h e) -> p h e", h=NH)
                    nc.vector.memset(v3[:, :, 64:128], 0.0)
                    nc.vector.memset(v3[:, :, 64:65], 1.0)
                    nc.vector.tensor_add(
                        v3[:, :, 0:64],
                        ps.rearrange("p (h e) -> p h e", e=64),
                        bv_t.rearrange("p (h e) -> p h e", e=64))

                for m in range(4, 8):
                    for j in range(4):
                        qk_unit(m, j)
                for m in range(4):
                    qk_unit(m, 0)
                for i in range(16):
                    v_unit(i)
                for j in range(1, 4):
                    for m in range(4):
                        qk_unit(m, j)

            # ---- phase C: attention, one head PAIR at a time ----
            # Heads 2p / 2p+1 live in partitions 0-63 / 64-127 of the qk
            # tiles, so their score matmuls run concurrently on PE row
            # tiles T0/T8 (64x128 mode, inferred from partition offsets).
            with tc.tile_pool(name="psS", bufs=1, space="PSUM") as psS, \
                 tc.tile_pool(name="psAV", bufs=2, space="PSUM") as psAV, \
                 tc.tile_pool(name="ppool", bufs=10) as ppool, \
                 tc.tile_pool(name="rpool", bufs=4) as rpool:
                for j in range(4):
                    js = slice(j * 512, (j + 1) * 512)
                    for p in range(NH // 2):
                        pts = [[], []]
                        for kk in range(8):
                            # head 2p on PE row tile T0, head 2p+1 on T8
                            ss0 = psS.tile([128, 1024], F32, tag="ss0", name="ss0")
                            ss1 = psS.tile([128, 1024], F32, tag="ss1", name="ss1")
                            for half in range(2):
                                kc = 2 * kk + half
                                ks = slice(kc * 128, (kc + 1) * 128)
                                hs = slice(half * 512, (half + 1) * 512)
                                _mm(nc, ss0[:, hs],
                                    qk_t[4 + p][0:64, ks],
                                    qk_t[p][0:64, js], True, True, mm_dt)
                                _mm(nc, ss1[:, hs],
                                    qk_t[4 + p][64:128, ks],
                                    qk_t[p][64:128, js], True, True, mm_dt)
                            pt0 = ppool.tile([128, 1024], sb_dt, tag="pt0", name="pt0")
                            pt1 = ppool.tile([128, 1024], sb_dt, tag="pt1", name="pt1")
                            if kk == 7:
                                # offload 2/16 of exp to DVE (Schraudolph bit
                                # trick): ACT is the attention pacer.
                                for ssx, ptx in ((ss0, pt0), (ss1, pt1)):
                                    tt = ppool.tile([128, 1024], F32,
                                                    tag="tt", name="tt", bufs=2)
                                    nc.vector.tensor_scalar(
                                        tt[:], ssx[:], EXP_A, EXP_B, Mult, Add)
                                    ti = ppool.tile([128, 1024], mybir.dt.int32,
                                                    tag="ti", name="ti", bufs=2)
                                    nc.vector.tensor_copy(ti[:], tt[:])
                                    nc.vector.tensor_copy(ptx[:], ti[:].bitcast(F32))
                            else:
                                nc.scalar.activation(pt0[:], ss0[:], Exp)
                                nc.scalar.activation(pt1[:], ss1[:], Exp)
                            pts[0].append(pt0)
                            pts[1].append(pt1)
                        avs = []
                        for t in range(2):
                            av = psAV.tile([128, 512], F32, tag=f"av{t}", name=f"av{t}")
                            h = 2 * p + t
                            for kc in range(16):
                                _mm(nc, av[:],
                                    v_t[kc][:, h * 128:h * 128 + 128],
                                    pts[t][kc // 2][:, (kc % 2) * 512:(kc % 2 + 1) * 512],
                                    kc == 0, kc == 15, mm_dt)
                            avs.append(av)
                        for t in range(2):
                            av = avs[t]
                            po = t * 64
                            z = rpool.tile([1, 512], F32, tag="z", name="z")
                            nc.vector.tensor_copy(z[:], av[64:65, :])
                            r = rpool.tile([1, 512], F32, tag="r", name="r")
                            nc.vector.reciprocal_approx_fast(out=r[:], in_=z[:])
                            rb = rpool.tile([64, 512], F32, tag="rb", name="rb")
                            nc.gpsimd.partition_broadcast(rb[:], r[:])
                            nc.vector.tensor_mul(
                                ao_t[p][po:po + 64, js],
                                av[0:64, :], rb[:])

                # ---- phase D: y^T = (w_proj stationary) @ ao^T ----
                # py shares the av tags/banks so proj(j) can run under the
                # ACT-bound attention of later q-blocks (no pool-stack WAR).
                for j in range(4):
                    js = slice(j * 512, (j + 1) * 512)
                    for m2 in range(8):
                        py = psAV.tile([128, 512], F32, tag=f"av{m2 % 2}",
                                       name="py")
                        for cl in range(4):
                            _mm(nc, py[:],
                                wp_t[cl][:, m2 * 128:(m2 + 1) * 128],
                                ao_t[cl][:, js],
                                cl == 0, cl == 3, mm_dt)
                        yt = ppool.tile([128, 512], F32, tag="yt", name="yt", bufs=4)
                        nc.vector.tensor_copy(yt[:], py[:])
                        nc.sync.dma_start(
                            out=yT[m2 * 128:(m2 + 1) * 128, js],
                            in_=yt[:])

    nc.compile()
    return nc


def make_in_maps(x, w_qkv, b_qkv, w_proj, mm_dt=MM_DT):
    np_dt = mybir.dt.np(BF16) if mm_dt == "bf16" else np.float32
    x = np.asarray(x, np.float32)
    w_qkv = np.asarray(w_qkv, np.float32)
    b_qkv = np.asarray(b_qkv, np.float32)
    w_proj = np.asarray(w_proj, np.float32)
    in_maps = []
    for c in range(NCORES):
        b, g = divmod(c, 2)
        h0 = g * NH
        qs = slice(h0 * D, h0 * D + CL)
        ks = slice(C + h0 * D, C + h0 * D + CL)
        vs = slice(2 * C + h0 * D, 2 * C + h0 * D + CL)
        wqk = np.concatenate([w_qkv[:, qs], w_qkv[:, ks]], axis=1)
        bq = b_qkv[qs] * SCALE
        bk = b_qkv[ks]
        bqk = np.concatenate([bq, bk]).reshape(8, 128).T  # [128, 8] col-chunks
        bv = np.broadcast_to(b_qkv[vs][None, :], (128, CL))
        in_maps.append({
            "xT": np.ascontiguousarray(x[b].T).astype(np_dt),
            "wqk": np.ascontiguousarray(wqk).astype(np_dt),
            "wv": np.ascontiguousarray(w_qkv[:, vs]).astype(np_dt),
            "wp": np.ascontiguousarray(w_proj[h0 * D:h0 * D + CL, :]).astype(np_dt),
            "bqk": np.ascontiguousarray(bqk, np.float32),
            "bv": np.ascontiguousarray(bv, np.float32),
        })
    return in_maps


def run(x, w_qkv, b_qkv, w_proj, b_proj, mm_dt=MM_DT, **spmd_kwargs):
    if mm_dt not in _CACHE:
        _CACHE[mm_dt] = build_nc(mm_dt)
    nc = _CACHE[mm_dt]
    in_maps = make_in_maps(x, w_qkv, b_qkv, w_proj, mm_dt)
    res = run_bass_kernel_spmd(nc, in_maps, core_ids=list(range(NCORES)),
                               **spmd_kwargs)
    b_proj = np.asarray(b_proj, np.float32)
    out = np.empty((B, N, C), np.float32)
    for b in range(B):
        acc = res.results[2 * b]["yT"] + res.results[2 * b + 1]["yT"]
        out[b] = acc.T + b_proj[None, :]
    return out, res


def kernel(x, w_qkv, b_qkv, w_proj, b_proj):
    out, _ = run(x, w_qkv, b_qkv, w_proj, b_proj)
    return out
